# revision 27
# baseline (speedup 1.0000x reference)
"""TRN2 Bass kernel for NetBackward: X = (I - A_{n-1}/n) @ ... @ (I - A_0/n).

Input  A: [1000, 512, 512] fp32.  Output X: [512, 512] fp32.

Distribution (8 NeuronCores, SPMD), contiguous-segment scan:
  - core c gets factors A[c*125:(c+1)*125]; local chain in transposed
    space: Y <- M_i^T Y (i descending) so A is the stationary operand in
    natural layout.
  - matmuls run in fp8e4m3 with perf_mode=DoubleRow: one instruction
    contracts K=256 (two 128-row planes), so a 512x512x512 step is 8
    matmuls (~225 ns each) instead of 16 fp32r ones (~250 ns each).
  - accuracy: master state lives in a persistent fp32 PSUM accumulator
    psum += A_i^T Y_r_i; the rounded operand is regenerated every
    SECOND step as Y_r = fp8(psum * (-1/n) + I) and shared by the step
    pair (the staleness only drops consecutive-pair cross terms,
    ~2e-4).  Only the rounded-operand cross terms see fp8 error, each
    suppressed by 1/n, so the partial product keeps ~2e-3 accuracy at
    fp8 speed.
  - A streams from HBM through the gpsimd cast-DMA (fp32 -> fp8e4m3,
    ~300-360 GB/s streaming), which is the roofline: the chain must
    read 131 MB of fp32 per core, so the step budget is ~3.4 us and
    the matmul + vector work hides under the DMA.
  - the per-pair transform runs as two 1024-col scalar_tensor_tensor
    halves on DVE (~1.5 us each), with the even step's first matmuls
    ordered so TensorE works while the second half is written.  (The
    ACT engine is useless here: activation ops measured ~4x slower
    than DVE for this, table-load dominated.)
  - each core ships 16*(Y_c - I) as fp8e4m3 (scaled out of the fp8
    denormal range); after an AllGather every core combines
    V_{j+1} = V_j + E_j^T V_j (V_0 = I) with DoubleRow matmuls, E_0
    recomputed from the local fp32 partial; core 0's output is
    returned.

Timing hooks (used by test.py only): build(chain_loop=L) wraps the
chain in a hardware For_i so per-chain time can be measured
differentially inside one NEFF; tail_loop does the same for the
gather+combine tail.
"""

import numpy as np

import concourse.mybir as mybir
from concourse import bacc
from concourse.bass_utils import run_bass_kernel_spmd
from concourse.tile import TileContext

dt = mybir.dt

N = 1000
D = 512
KB = D // 128
NCORES = 8
SEG = N // NCORES
DR = mybir.MatmulPerfMode.DoubleRow
COPY = mybir.ActivationFunctionType.Copy


def build(seg=SEG, n_total=N, with_combine=True, chain_loop=1, tail_loop=1, ag_reps=1):
    scale = -1.0 / float(n_total)
    nc = bacc.Bacc()
    a = nc.dram_tensor("a", [seg, D, D], dt.float32, kind="ExternalInput")
    out = nc.dram_tensor("out", [D, D], dt.float32, kind="ExternalOutput")

    # blocked identity: eye_blk[p, kb*D + m] = I[kb*128 + p, m]
    eye = np.eye(D, dtype=np.float32)
    eye_blk = eye.reshape(KB, 128, D).transpose(1, 0, 2).reshape(128, KB * D)
    eye_dram = nc.inline_tensor(eye_blk, name="eye_blk")

    y_loc = nc.dram_tensor("y_loc", [D, D], dt.float8e4)
    y_all = nc.dram_tensor("y_all", [NCORES, D, D], dt.float8e4, addr_space="Shared")

    a_v = a.rearrange("s (kb p) m -> s p kb m", p=128)

    def emit_chain(tc, a_pool, a32_pool, y_pool, ps_big, eye_f, y0_r):
        # Steps are processed in pairs sharing one rounded operand: the
        # fp8 operand regenerated after step s-1 serves steps s and s+1.
        # The extra staleness only drops consecutive-pair cross terms
        # (~2e-4 in the partial), and halves the DVE PSUM-read traffic.
        # A streams as raw fp32 on the plain sync-DMA queue (faster than
        # the gpsimd cast-DMA) and DVE does the fp32->fp8 cast; the
        # casts for pair k+1 are emitted before pair k's transform so
        # they never delay the next pair's matmuls.  The transform is
        # split into two 1024-col halves and the first matmuls of an
        # even step are ordered (mb0/1, t=0 first) so TensorE keeps
        # working while the second half is still being written.
        def load(i):
            at32 = a32_pool.tile([128, KB * D], dt.float32, tag="a32", name=f"a32_{i}")
            at32_3 = at32[:].rearrange("p (kb m) -> p kb m", m=D)
            nc.sync.dma_start(out=at32_3, in_=a_v[seg - 1 - i])
            at8 = a_pool.tile([128, KB * D], dt.float8e4, tag="a", name=f"a{i}")
            nc.vector.tensor_copy(out=at8[:], in_=at32[:])
            return at8

        tiles = {}
        for i in (0, 1):
            if i < seg:
                tiles[i] = load(i)
        y_cur = y0_r
        for i in range(seg):
            if i % 2 == 0:
                for nxt in (i + 2, i + 3):
                    if nxt < seg:
                        tiles[nxt] = load(nxt)
            at3 = tiles.pop(i)[:].rearrange("p (kb m) -> p kb m", m=D)
            y3 = y_cur[:].rearrange("p (kb m) -> p kb m", m=D)
            order = (
                [(0, 0), (1, 0), (0, 1), (1, 1), (2, 0), (2, 1), (3, 0), (3, 1)]
                if i % 2 == 0
                else [(mb, t) for mb in range(KB) for t in (0, 1)]
            )
            for mb, t in order:
                nc.tensor.matmul(
                    ps_big[:, mb * D : (mb + 1) * D],
                    at3[:, 2 * t : 2 * t + 2, mb * 128 : mb * 128 + 128],
                    y3[:, 2 * t : 2 * t + 2, :],
                    start=(i == 0 and t == 0),
                    stop=(i == seg - 1 and t == 1),
                    perf_mode=DR,
                    skip_group_check=True,
                )
            if i % 2 == 1 and i + 1 < seg:
                y_new = y_pool.tile(
                    [128, KB * D], dt.float8e4, tag="y", name=f"y{i}"
                )
                for h in (0, 1):
                    nc.vector.scalar_tensor_tensor(
                        out=y_new[:, h * 2 * D : (h + 1) * 2 * D],
                        in0=ps_big[:, h * 2 * D : (h + 1) * 2 * D],
                        scalar=scale,
                        in1=eye_f[:, h * 2 * D : (h + 1) * 2 * D],
                        op0=mybir.AluOpType.mult,
                        op1=mybir.AluOpType.add,
                    )
                y_cur = y_new

    with TileContext(nc) as tc:
        with (
            tc.tile_pool(name="y", bufs=3) as y_pool,
            tc.tile_pool(name="a", bufs=6) as a_pool,
            tc.tile_pool(name="a32", bufs=5) as a32_pool,
            tc.tile_pool(name="acc", bufs=1, space="PSUM") as acc_pool,
            tc.tile_pool(name="misc", bufs=1) as misc_pool,
        ):
            eye_f = misc_pool.tile([128, KB * D], dt.float32, tag="eyef")
            nc.sync.dma_start(out=eye_f[:], in_=eye_dram[:])
            y0_r = misc_pool.tile([128, KB * D], dt.float8e4, tag="y0")
            nc.gpsimd.dma_start(out=y0_r[:], in_=eye_dram[:])

            ps_big = acc_pool.tile([128, KB * D], dt.float32, tag="acc", name="acc")

            if chain_loop == 1:
                emit_chain(tc, a_pool, a32_pool, y_pool, ps_big[:], eye_f, y0_r)
            else:
                with tc.For_i(0, chain_loop, 1):
                    emit_chain(tc, a_pool, a32_pool, y_pool, ps_big[:], eye_f, y0_r)

            # full-precision local partial: Y = psum*scale + I (fp32)
            y_fin = misc_pool.tile([128, KB * D], dt.float32, tag="yfin")
            nc.vector.scalar_tensor_tensor(
                out=y_fin[:],
                in0=ps_big[:],
                scalar=scale,
                in1=eye_f[:],
                op0=mybir.AluOpType.mult,
                op1=mybir.AluOpType.add,
            )

            if not with_combine:
                out_v = out.rearrange("(kb p) m -> p kb m", p=128)
                nc.sync.dma_start(
                    out=out_v, in_=y_fin[:].rearrange("p (kb m) -> p kb m", m=D)
                )
                nc.compile()
                return nc

            # ---- tail: each core computes 16*(Y_local - I) in fp32 and
            # ships it as fp8e4m3 (E is ~1e-2 magnitude; the 16x scaling
            # lifts it out of the fp8 denormal range so relative accuracy
            # stays ~3%; quantizing Y itself would cost 6e-2 on the unit
            # diagonal).  AllGather the 8 fp8 E_j, then combine
            # V_{j+1} = V_j + E_j^T V_j, V_0 = I with DoubleRow matmuls
            # on the 16x-scaled operands (products carry 256x, undone in
            # the PSUM->V transform).  E_0 is recomputed from the local
            # fp32 partial, so core 0's X is exact; other cores' outputs
            # are discarded. ----
            ESC = 16.0
            eye16 = misc_pool.tile([128, KB * D], dt.float32, tag="eye16")
            nc.vector.tensor_scalar(
                out=eye16[:],
                in0=eye_f[:],
                scalar1=ESC,
                scalar2=None,
                op0=mybir.AluOpType.mult,
            )
            e_loc8 = misc_pool.tile([128, KB * D], dt.float8e4, tag="el8")
            nc.vector.scalar_tensor_tensor(
                out=e_loc8[:],
                in0=y_fin[:],
                scalar=ESC,
                in1=eye16[:],
                op0=mybir.AluOpType.mult,
                op1=mybir.AluOpType.subtract,
            )
            y_loc_v = y_loc.rearrange("(kb p) m -> p kb m", p=128)
            y_all_v = y_all.rearrange("c (kb p) m -> c p kb m", p=128)

            eye16_8 = misc_pool.tile([128, KB * D], dt.float8e4, tag="ey168")
            nc.vector.tensor_copy(out=eye16_8[:], in_=eye16[:])

            # collectives inside For_i desync the mesh, so the gather is
            # amplified by static unrolling (ag_reps) instead
            for _ in range(ag_reps):
                nc.sync.dma_start(
                    out=y_loc_v, in_=e_loc8[:].rearrange("p (kb m) -> p kb m", m=D)
                )
                nc.gpsimd.collective_compute(
                    "AllGather",
                    mybir.AluOpType.bypass,
                    ins=[y_loc[:]],
                    outs=[y_all[:]],
                    replica_groups=[list(range(NCORES))],
                )

            def emit_tail():
                # prefetch all gathered E_j up front (they only depend on
                # the AllGather)
                ejs = [None] * NCORES
                for j in range(1, NCORES):
                    ej = a_pool.tile(
                        [128, KB * D], dt.float8e4, tag="a", name=f"ej{j}"
                    )
                    ej3 = ej[:].rearrange("p (kb m) -> p kb m", m=D)
                    nc.sync.dma_start(out=ej3, in_=y_all_v[j])
                    ejs[j] = ej
                v_r = eye16_8
                for j in range(NCORES):
                    ej = e_loc8 if j == 0 else ejs[j]
                    ej3 = ej[:].rearrange("p (kb m) -> p kb m", m=D)
                    v3 = v_r[:].rearrange("p (kb m) -> p kb m", m=D)
                    # mb0/1 t0 first: they only need the first half of the
                    # previous v_new transform
                    order = [(0, 0), (1, 0), (0, 1), (1, 1), (2, 0), (2, 1), (3, 0), (3, 1)]
                    for mb, t in order:
                        nc.tensor.matmul(
                            ps_big[:, mb * D : (mb + 1) * D],
                            ej3[:, 2 * t : 2 * t + 2, mb * 128 : mb * 128 + 128],
                            v3[:, 2 * t : 2 * t + 2, :],
                            start=(j == 0 and t == 0),
                            stop=(j == NCORES - 1 and t == 1),
                            perf_mode=DR,
                            skip_group_check=True,
                        )
                    if j < NCORES - 1:
                        v_new = y_pool.tile(
                            [128, KB * D], dt.float8e4, tag="y", name=f"v{j}"
                        )
                        for h in (0, 1):
                            nc.vector.scalar_tensor_tensor(
                                out=v_new[:, h * 2 * D : (h + 1) * 2 * D],
                                in0=ps_big[:, h * 2 * D : (h + 1) * 2 * D],
                                scalar=1.0 / ESC,
                                in1=eye16[:, h * 2 * D : (h + 1) * 2 * D],
                                op0=mybir.AluOpType.mult,
                                op1=mybir.AluOpType.add,
                            )
                        v_r = v_new

            if tail_loop == 1:
                emit_tail()
            else:
                with tc.For_i(0, tail_loop, 1):
                    emit_tail()

            x_fin = misc_pool.tile([128, KB * D], dt.float32, tag="xfin")
            nc.vector.scalar_tensor_tensor(
                out=x_fin[:],
                in0=ps_big[:],
                scalar=1.0 / (ESC * ESC),
                in1=eye_f[:],
                op0=mybir.AluOpType.mult,
                op1=mybir.AluOpType.add,
            )
            out_v = out.rearrange("(kb p) m -> p kb m", p=128)
            nc.sync.dma_start(
                out=out_v, in_=x_fin[:].rearrange("p (kb m) -> p kb m", m=D)
            )

    nc.compile()
    return nc


_NC_CACHE = None


def kernel(A: np.ndarray) -> np.ndarray:
    global _NC_CACHE
    A = np.ascontiguousarray(np.asarray(A, dtype=np.float32))
    assert A.shape == (N, D, D), A.shape

    if _NC_CACHE is None:
        _NC_CACHE = build()
    nc = _NC_CACHE

    in_maps = [{"a": A[c * SEG : (c + 1) * SEG]} for c in range(NCORES)]
    res = run_bass_kernel_spmd(nc, in_maps, list(range(NCORES)))
    return np.asarray(res.results[0]["out"], dtype=np.float32)


# revision 29
# speedup vs baseline: 1.1708x; 1.1708x over previous
"""TRN2 Bass kernel for NetBackward: X = (I - A_{n-1}/n) @ ... @ (I - A_0/n).

Input  A: [1000, 512, 512] fp32.  Output X: [512, 512] fp32.

Distribution (8 NeuronCores, SPMD), contiguous-segment scan:
  - core c gets factors A[c*125:(c+1)*125]; local chain in transposed
    space: Y <- M_i^T Y (i descending) so A is the stationary operand in
    natural layout.
  - matmuls run in fp8e4m3 with perf_mode=DoubleRow: one instruction
    contracts K=256 (two 128-row planes), so a 512x512x512 step is 8
    matmuls (~225 ns each) instead of 16 fp32r ones (~250 ns each).
  - accuracy: master state lives in a persistent fp32 PSUM accumulator
    psum += A_i^T Y_r_i; the rounded operand is regenerated every
    SECOND step as Y_r = fp8(psum * (-1/n) + I) and shared by the step
    pair (the staleness only drops consecutive-pair cross terms,
    ~2e-4).  Only the rounded-operand cross terms see fp8 error, each
    suppressed by 1/n, so the partial product keeps ~2e-3 accuracy at
    fp8 speed.
  - A streams from HBM through the gpsimd cast-DMA (fp32 -> fp8e4m3,
    ~300-360 GB/s streaming), which is the roofline: the chain must
    read 131 MB of fp32 per core, so the step budget is ~3.4 us and
    the matmul + vector work hides under the DMA.
  - the per-pair transform runs as two 1024-col scalar_tensor_tensor
    halves on DVE (~1.5 us each), with the even step's first matmuls
    ordered so TensorE works while the second half is written.  (The
    ACT engine is useless here: activation ops measured ~4x slower
    than DVE for this, table-load dominated.)
  - each core ships 16*(Y_c - I) as fp8e4m3 (scaled out of the fp8
    denormal range); after an AllGather every core combines
    V_{j+1} = V_j + E_j^T V_j (V_0 = I) with DoubleRow matmuls, E_0
    recomputed from the local fp32 partial; core 0's output is
    returned.

Timing hooks (used by test.py only): build(chain_loop=L) wraps the
chain in a hardware For_i so per-chain time can be measured
differentially inside one NEFF; tail_loop does the same for the
gather+combine tail.
"""

import numpy as np

import concourse.mybir as mybir
from concourse import bacc
from concourse.bass_utils import run_bass_kernel_spmd
from concourse.tile import TileContext

dt = mybir.dt

N = 1000
D = 512
KB = D // 128
NCORES = 8
SEG = N // NCORES
DR = mybir.MatmulPerfMode.DoubleRow
COPY = mybir.ActivationFunctionType.Copy


def build(seg=SEG, n_total=N, with_combine=True, chain_loop=1, tail_loop=1, ag_reps=1):
    scale = -1.0 / float(n_total)
    nc = bacc.Bacc()
    a = nc.dram_tensor("a", [seg, D, D], dt.float32, kind="ExternalInput")
    out = nc.dram_tensor("out", [D, D], dt.float32, kind="ExternalOutput")

    # blocked identity: eye_blk[p, kb*D + m] = I[kb*128 + p, m]
    eye = np.eye(D, dtype=np.float32)
    eye_blk = eye.reshape(KB, 128, D).transpose(1, 0, 2).reshape(128, KB * D)
    eye_dram = nc.inline_tensor(eye_blk, name="eye_blk")

    y_loc = nc.dram_tensor("y_loc", [D, D], dt.float8e4)
    y_all = nc.dram_tensor("y_all", [NCORES, D, D], dt.float8e4, addr_space="Shared")

    a_v = a.rearrange("s (kb p) m -> s p kb m", p=128)

    def emit_chain(tc, a_pool, a32_pool, y_pool, ps_big, eye_f, y0_r):
        # Steps are processed in pairs sharing one rounded operand: the
        # fp8 operand regenerated after step s-1 serves steps s and s+1.
        # The extra staleness only drops consecutive-pair cross terms
        # (~2e-4 in the partial), and halves the DVE PSUM-read traffic.
        # A streams through the gpsimd cast-DMA (fp32 -> fp8e4m3); a
        # plain-DMA + DVE-cast variant measured ~8% slower at full scale
        # (the casts displace the transform halves in the in-order DVE
        # queue and delay TensorE).  The transform is split into two
        # 1024-col halves and the first matmuls of an even step are
        # ordered (mb0/1, t=0 first) so TensorE keeps working while the
        # second half is still being written.
        def load(i):
            at8 = a_pool.tile([128, KB * D], dt.float8e4, tag="a", name=f"a{i}")
            at8_3 = at8[:].rearrange("p (kb m) -> p kb m", m=D)
            nc.gpsimd.dma_start(out=at8_3, in_=a_v[seg - 1 - i])
            return at8

        tiles = {}
        for i in (0, 1):
            if i < seg:
                tiles[i] = load(i)
        y_cur = y0_r
        for i in range(seg):
            if i % 2 == 0:
                for nxt in (i + 2, i + 3):
                    if nxt < seg:
                        tiles[nxt] = load(nxt)
            at3 = tiles.pop(i)[:].rearrange("p (kb m) -> p kb m", m=D)
            y3 = y_cur[:].rearrange("p (kb m) -> p kb m", m=D)
            order = (
                [(0, 0), (1, 0), (0, 1), (1, 1), (2, 0), (2, 1), (3, 0), (3, 1)]
                if i % 2 == 0
                else [(mb, t) for mb in range(KB) for t in (0, 1)]
            )
            for mb, t in order:
                nc.tensor.matmul(
                    ps_big[:, mb * D : (mb + 1) * D],
                    at3[:, 2 * t : 2 * t + 2, mb * 128 : mb * 128 + 128],
                    y3[:, 2 * t : 2 * t + 2, :],
                    start=(i == 0 and t == 0),
                    stop=(i == seg - 1 and t == 1),
                    perf_mode=DR,
                    skip_group_check=True,
                )
            if i % 2 == 1 and i + 1 < seg:
                y_new = y_pool.tile(
                    [128, KB * D], dt.float8e4, tag="y", name=f"y{i}"
                )
                for h in (0, 1):
                    nc.vector.scalar_tensor_tensor(
                        out=y_new[:, h * 2 * D : (h + 1) * 2 * D],
                        in0=ps_big[:, h * 2 * D : (h + 1) * 2 * D],
                        scalar=scale,
                        in1=eye_f[:, h * 2 * D : (h + 1) * 2 * D],
                        op0=mybir.AluOpType.mult,
                        op1=mybir.AluOpType.add,
                    )
                y_cur = y_new

    with TileContext(nc) as tc:
        with (
            tc.tile_pool(name="y", bufs=3) as y_pool,
            tc.tile_pool(name="a", bufs=6) as a_pool,
            tc.tile_pool(name="a32", bufs=5) as a32_pool,
            tc.tile_pool(name="acc", bufs=1, space="PSUM") as acc_pool,
            tc.tile_pool(name="misc", bufs=1) as misc_pool,
        ):
            eye_f = misc_pool.tile([128, KB * D], dt.float32, tag="eyef")
            nc.sync.dma_start(out=eye_f[:], in_=eye_dram[:])
            y0_r = misc_pool.tile([128, KB * D], dt.float8e4, tag="y0")
            nc.gpsimd.dma_start(out=y0_r[:], in_=eye_dram[:])

            ps_big = acc_pool.tile([128, KB * D], dt.float32, tag="acc", name="acc")

            if chain_loop == 1:
                emit_chain(tc, a_pool, a32_pool, y_pool, ps_big[:], eye_f, y0_r)
            else:
                with tc.For_i(0, chain_loop, 1):
                    emit_chain(tc, a_pool, a32_pool, y_pool, ps_big[:], eye_f, y0_r)

            # full-precision local partial: Y = psum*scale + I (fp32)
            y_fin = misc_pool.tile([128, KB * D], dt.float32, tag="yfin")
            nc.vector.scalar_tensor_tensor(
                out=y_fin[:],
                in0=ps_big[:],
                scalar=scale,
                in1=eye_f[:],
                op0=mybir.AluOpType.mult,
                op1=mybir.AluOpType.add,
            )

            if not with_combine:
                out_v = out.rearrange("(kb p) m -> p kb m", p=128)
                nc.sync.dma_start(
                    out=out_v, in_=y_fin[:].rearrange("p (kb m) -> p kb m", m=D)
                )
                nc.compile()
                return nc

            # ---- tail: each core computes 16*(Y_local - I) in fp32 and
            # ships it as fp8e4m3 (E is ~1e-2 magnitude; the 16x scaling
            # lifts it out of the fp8 denormal range so relative accuracy
            # stays ~3%; quantizing Y itself would cost 6e-2 on the unit
            # diagonal).  AllGather the 8 fp8 E_j, then combine
            # V_{j+1} = V_j + E_j^T V_j, V_0 = I with DoubleRow matmuls
            # on the 16x-scaled operands (products carry 256x, undone in
            # the PSUM->V transform).  E_0 is recomputed from the local
            # fp32 partial, so core 0's X is exact; other cores' outputs
            # are discarded. ----
            ESC = 16.0
            eye16 = misc_pool.tile([128, KB * D], dt.float32, tag="eye16")
            nc.vector.tensor_scalar(
                out=eye16[:],
                in0=eye_f[:],
                scalar1=ESC,
                scalar2=None,
                op0=mybir.AluOpType.mult,
            )
            e_loc8 = misc_pool.tile([128, KB * D], dt.float8e4, tag="el8")
            nc.vector.scalar_tensor_tensor(
                out=e_loc8[:],
                in0=y_fin[:],
                scalar=ESC,
                in1=eye16[:],
                op0=mybir.AluOpType.mult,
                op1=mybir.AluOpType.subtract,
            )
            y_loc_v = y_loc.rearrange("(kb p) m -> p kb m", p=128)
            y_all_v = y_all.rearrange("c (kb p) m -> c p kb m", p=128)

            eye16_8 = misc_pool.tile([128, KB * D], dt.float8e4, tag="ey168")
            nc.vector.tensor_copy(out=eye16_8[:], in_=eye16[:])

            # collectives inside For_i desync the mesh, so the gather is
            # amplified by static unrolling (ag_reps) instead
            for _ in range(ag_reps):
                nc.sync.dma_start(
                    out=y_loc_v, in_=e_loc8[:].rearrange("p (kb m) -> p kb m", m=D)
                )
                nc.gpsimd.collective_compute(
                    "AllGather",
                    mybir.AluOpType.bypass,
                    ins=[y_loc[:]],
                    outs=[y_all[:]],
                    replica_groups=[list(range(NCORES))],
                )

            def emit_tail():
                # prefetch all gathered E_j up front (they only depend on
                # the AllGather)
                ejs = [None] * NCORES
                for j in range(1, NCORES):
                    ej = a_pool.tile(
                        [128, KB * D], dt.float8e4, tag="a", name=f"ej{j}"
                    )
                    ej3 = ej[:].rearrange("p (kb m) -> p kb m", m=D)
                    nc.sync.dma_start(out=ej3, in_=y_all_v[j])
                    ejs[j] = ej
                v_r = eye16_8
                for j in range(NCORES):
                    ej = e_loc8 if j == 0 else ejs[j]
                    ej3 = ej[:].rearrange("p (kb m) -> p kb m", m=D)
                    v3 = v_r[:].rearrange("p (kb m) -> p kb m", m=D)
                    # mb0/1 t0 first: they only need the first half of the
                    # previous v_new transform
                    order = [(0, 0), (1, 0), (0, 1), (1, 1), (2, 0), (2, 1), (3, 0), (3, 1)]
                    for mb, t in order:
                        nc.tensor.matmul(
                            ps_big[:, mb * D : (mb + 1) * D],
                            ej3[:, 2 * t : 2 * t + 2, mb * 128 : mb * 128 + 128],
                            v3[:, 2 * t : 2 * t + 2, :],
                            start=(j == 0 and t == 0),
                            stop=(j == NCORES - 1 and t == 1),
                            perf_mode=DR,
                            skip_group_check=True,
                        )
                    if j < NCORES - 1:
                        v_new = y_pool.tile(
                            [128, KB * D], dt.float8e4, tag="y", name=f"v{j}"
                        )
                        for h in (0, 1):
                            nc.vector.scalar_tensor_tensor(
                                out=v_new[:, h * 2 * D : (h + 1) * 2 * D],
                                in0=ps_big[:, h * 2 * D : (h + 1) * 2 * D],
                                scalar=1.0 / ESC,
                                in1=eye16[:, h * 2 * D : (h + 1) * 2 * D],
                                op0=mybir.AluOpType.mult,
                                op1=mybir.AluOpType.add,
                            )
                        v_r = v_new

            if tail_loop == 1:
                emit_tail()
            else:
                with tc.For_i(0, tail_loop, 1):
                    emit_tail()

            x_fin = misc_pool.tile([128, KB * D], dt.float32, tag="xfin")
            nc.vector.scalar_tensor_tensor(
                out=x_fin[:],
                in0=ps_big[:],
                scalar=1.0 / (ESC * ESC),
                in1=eye_f[:],
                op0=mybir.AluOpType.mult,
                op1=mybir.AluOpType.add,
            )
            out_v = out.rearrange("(kb p) m -> p kb m", p=128)
            nc.sync.dma_start(
                out=out_v, in_=x_fin[:].rearrange("p (kb m) -> p kb m", m=D)
            )

    nc.compile()
    return nc


_NC_CACHE = None


def kernel(A: np.ndarray) -> np.ndarray:
    global _NC_CACHE
    A = np.ascontiguousarray(np.asarray(A, dtype=np.float32))
    assert A.shape == (N, D, D), A.shape

    if _NC_CACHE is None:
        _NC_CACHE = build()
    nc = _NC_CACHE

    in_maps = [{"a": A[c * SEG : (c + 1) * SEG]} for c in range(NCORES)]
    res = run_bass_kernel_spmd(nc, in_maps, list(range(NCORES)))
    return np.asarray(res.results[0]["out"], dtype=np.float32)
